# revision 9
# baseline (speedup 1.0000x reference)
"""Trainium2 Bass kernel for nn_MultiHeadModel (segment_reduce), 8-core SPMD.

Math (reference):
    xp  = x @ Wp + bp                              # [N, 256]
    class_emb[g] = (sum_{i in g} m_i * xp_i) / n_g # [G, 256]  (segment mean)
    h   = concat(repeat(class_emb, C), xp[idx])    # [G*C, 512]
    out = relu(relu(h@W1+b1)@W2+b2) @ W3 + b3      # [G*C, 1]
(edge_attr projection in the reference is dead code - output never uses it.)

Structure:
  *  sum_g m_i*xp_i = (sum_g m_i*x_i) @ Wp + n_g*bp: the [N,256] projection
     is never materialized. A one-hot indicator matmul does the segment sum
     over raw x rows on the PE; only [128,256] per core gets projected.
  *  batch is sorted -> graphs shard contiguously: core k owns graphs
     [128k,128k+128). Host drops masked-out rows and packs x plus the
     one-hot indicator into one partition-major stream tensor, 4 node
     tiles per DMA.
  *  xp[idx] rows are host-gathered per core (transposed); the device
     projects them. Those matmuls AND the h1 pre-activation matmuls are
     interleaved into the segment-sum stream so the PE never idles (keeps
     the HAM clock un-throttled). Only the class-embedding broadcast-add,
     relu, h2 and the output head run after the stream.
  *  repeat(class_emb, C) @ W1_top == repeat(class_emb @ W1_top, C),
     applied with a step-0 free-dim AP broadcast.
  *  All weights/biases/identity/gather rows arrive in ONE packed const
     DMA ([128, ~7.8k] partition-major, f32 views via bitcast).
  *  Matmuls in float32r (fp32_mode=HIGH single pass, ~2 cyc/row,
     ~1.5e-4 relative error). All activations computed transposed
     (features on partitions) - no transposes except one 128x256 tile.
"""
import numpy as np
from contextlib import ExitStack

import concourse.bacc as bacc
import concourse.mybir as mybir
from concourse.tile import TileContext
from concourse.bass_utils import run_bass_kernel_spmd

M = 8                 # cores
G = 1024              # graphs
C = 16                # classes
GL = G // M           # graphs per core (128)
D = 256
D2 = 512
ROWS = G * C // M     # MLP rows per core (2048)
NCH = ROWS // 512     # 512-wide row chunks (4)
FW = D + 1            # stream row width: 256 x-feats + 1 graph-id (257)
SUP = 4               # node-tiles per stream DMA

f32 = mybir.dt.float32
f32r = mybir.dt.float32r
Relu = mybir.ActivationFunctionType.Relu
Copy = mybir.ActivationFunctionType.Copy

# ---- packed constant layout (columns of a [128, CW] tile) --------------
_off = {}
_c = 0
def _span(name, w):
    global _c
    _off[name] = (_c, w)
    _c += w
for _k in range(2):
    _span(f"wp{_k}", D)
for _k in range(2):
    _span(f"w1t{_k}", D2)
for _k in range(2):
    _span(f"w1b{_k}", D2)
for _k in range(4):
    _span(f"w2{_k}", D)
for _k in range(2):
    _span(f"w3{_k}", 1)
for _k in range(2):
    _span(f"bp{_k}", 1)
for _k in range(4):
    _span(f"b1{_k}", 1)
for _k in range(2):
    _span(f"b2{_k}", 1)
_span("b3", 1)
_span("inv", 1)
_span("ident", 128)
for _k in range(2):
    _span(f"xg{_k}", ROWS)
CW = _c

_cache = {}


def _build(NT):
    NS = (NT + SUP - 1) // SUP
    nc = bacc.Bacc(None, target_bir_lowering=False, debug=False)
    xci = nc.dram_tensor("xci", [128, NT * FW], f32r, kind="ExternalInput")
    iot = nc.dram_tensor("iot", [128, GL], f32r, kind="ExternalInput")
    cpk = nc.dram_tensor("cpk", [128, CW], f32r, kind="ExternalInput")
    out = nc.dram_tensor("out", [1, ROWS], f32, kind="ExternalOutput")

    with TileContext(nc) as tc, ExitStack() as ctx:
        cst = ctx.enter_context(tc.tile_pool(name="cst", bufs=1))
        stream = ctx.enter_context(tc.tile_pool(name="stream", bufs=6))
        pseg = ctx.enter_context(tc.tile_pool(name="pseg", bufs=1, space="PSUM"))
        pmisc = ctx.enter_context(tc.tile_pool(name="pmisc", bufs=2, space="PSUM"))
        pml = ctx.enter_context(tc.tile_pool(name="pml", bufs=4, space="PSUM"))

        # ---- stream head first (segment-sum can start immediately), then
        # the packed const DMA, then the rest of the stream ---------------
        def stream_dma(st):
            t0 = st * SUP
            n_sub = min(SUP, NT - t0)
            stile = stream.tile([128, SUP * FW], f32r, tag="s")
            eng = nc.sync if st % 2 == 0 else nc.scalar
            eng.dma_start(out=stile[:, :n_sub * FW],
                          in_=xci[:, t0 * FW:(t0 + n_sub) * FW])
            return stile, n_sub

        iota_t = cst.tile([128, GL], f32r, tag="iota")
        nc.sync.dma_start(out=iota_t[:], in_=iot[:])

        NSH = min(2, NS)
        head = [stream_dma(st) for st in range(NSH)]

        ctile = cst.tile([128, CW], f32r, tag="cpk")
        nc.sync.dma_start(out=ctile[:], in_=cpk[:])

        def cs(name, dt=f32r):           # const slice
            o, w = _off[name]
            ap = ctile[:, o:o + w]
            return ap.bitcast(dt) if dt is not f32r else ap

        # ---- stream + interleaved MLP-left matmuls ---------------------
        psum_sx = pseg.tile([GL, D], f32)
        xgp = [[None] * NCH for _ in range(2)]
        h1pre = [[None] * NCH for _ in range(4)]

        def xgp_group(mc, n):
            pp = pml.tile([128, 512], f32, tag="mlp")
            for k2 in range(2):
                nc.tensor.matmul(out=pp[:],
                                 lhsT=cs(f"wp{k2}")[:, mc * 128:(mc + 1) * 128],
                                 rhs=cs(f"xg{k2}")[:, n * 512:(n + 1) * 512],
                                 start=(k2 == 0), stop=(k2 == 1))
            t = cst.tile([128, 512], f32r, tag=f"xgp{mc}{n}")
            nc.vector.tensor_scalar_add(out=t[:], in0=pp[:],
                                        scalar1=cs(f"bp{mc}", f32)[:, :1])
            xgp[mc][n] = t

        def h1pre_group(m1, n):
            ph = pml.tile([128, 512], f32, tag="mlp")
            for k2 in range(2):
                nc.tensor.matmul(out=ph[:],
                                 lhsT=cs(f"w1b{k2}")[:, m1 * 128:(m1 + 1) * 128],
                                 rhs=xgp[k2][n][:],
                                 start=(k2 == 0), stop=(k2 == 1))
            t = cst.tile([128, 512], f32, tag=f"h1p{m1}{n}")
            nc.scalar.activation(out=t[:], in_=ph[:], func=Copy)
            h1pre[m1][n] = t

        jobs = []
        for n in range(NCH):
            jobs.append((xgp_group, 0, n))
            jobs.append((xgp_group, 1, n))
            for m1 in range(4):
                jobs.append((h1pre_group, m1, n))
        jobs_iter = iter(jobs)

        for st in range(NS):
            if st < NSH:
                stile, n_sub = head[st]
            else:
                stile, n_sub = stream_dma(st)
            for s in range(n_sub):
                t = st * SUP + s
                # one-hot indicator from the graph-id column (exact 0/1)
                ind_t = stream.tile([128, GL], f32r, tag="ind")
                nc.vector.tensor_tensor(
                    out=ind_t[:],
                    in0=stile[:, s * FW + D:s * FW + D + 1].to_broadcast([128, GL]),
                    in1=iota_t[:],
                    op=mybir.AluOpType.is_equal,
                )
                nc.tensor.matmul(out=psum_sx[:],
                                 lhsT=ind_t[:],
                                 rhs=stile[:, s * FW:s * FW + D],
                                 start=(t == 0), stop=(t == NT - 1))
            if st >= NS - 8:
                for _ in range(3):
                    job = next(jobs_iter, None)
                    if job:
                        job[0](job[1], job[2])
        for job in jobs_iter:
            job[0](job[1], job[2])

        # ---- class-embedding chain (small, latency-bound) --------------
        sxs = cst.tile([GL, D], f32, tag="sxs")
        nc.vector.tensor_scalar_mul(out=sxs[:], in0=psum_sx[:],
                                    scalar1=cs("inv", f32)[:, :1])
        sxT = []
        for c2 in range(2):
            pt = pmisc.tile([128, 128], f32, tag="mm")
            nc.tensor.transpose(out=pt[:], in_=sxs[:, c2 * 128:(c2 + 1) * 128],
                                identity=cs("ident", f32))
            st_ = cst.tile([128, 128], f32r, tag=f"sxT{c2}")
            nc.vector.tensor_copy(out=st_[:], in_=pt[:])
            sxT.append(st_)
        clsembT = []
        for mc in range(2):
            pc = pmisc.tile([128, 128], f32, tag="mm")
            for k2 in range(2):
                nc.tensor.matmul(out=pc[:],
                                 lhsT=cs(f"wp{k2}")[:, mc * 128:(mc + 1) * 128],
                                 rhs=sxT[k2][:], start=(k2 == 0), stop=(k2 == 1))
            ce = cst.tile([128, 128], f32r, tag=f"ce{mc}")
            nc.vector.tensor_scalar_add(out=ce[:], in0=pc[:],
                                        scalar1=cs(f"bp{mc}", f32)[:, :1])
            clsembT.append(ce)
        cls1b = []
        for m1 in range(4):
            p1_ = pmisc.tile([128, 128], f32, tag="mm")
            for k2 in range(2):
                nc.tensor.matmul(out=p1_[:],
                                 lhsT=cs(f"w1t{k2}")[:, m1 * 128:(m1 + 1) * 128],
                                 rhs=clsembT[k2][:], start=(k2 == 0), stop=(k2 == 1))
            cb = cst.tile([128, GL], f32, tag=f"cb{m1}")
            nc.vector.tensor_scalar_add(out=cb[:], in0=p1_[:],
                                        scalar1=cs(f"b1{m1}", f32)[:, :1])
            cls1b.append(cb)

        # ---- tail: h1 = relu(h1pre + cls1b[bcast]), h2, out, pipelined -
        out_sb = cst.tile([1, ROWS], f32, tag="osb")
        for n in range(NCH):
            h1n = []
            for m1 in range(4):
                hp = h1pre[m1][n]
                nc.vector.tensor_tensor(
                    out=hp[:].rearrange("p (g c) -> p g c", c=C),
                    in0=hp[:].rearrange("p (g c) -> p g c", c=C),
                    in1=cls1b[m1][:, n * 32:(n + 1) * 32, None].to_broadcast([128, 32, C]),
                    op=mybir.AluOpType.add,
                )
                h = cst.tile([128, 512], f32r, tag=f"h1{m1}{n}")
                nc.scalar.activation(out=h[:], in_=hp[:], func=Relu)
                h1n.append(h)
            h2n = []
            for m2 in range(2):
                ph2 = pml.tile([128, 512], f32, tag="mlp")
                for k4 in range(4):
                    nc.tensor.matmul(out=ph2[:],
                                     lhsT=cs(f"w2{k4}")[:, m2 * 128:(m2 + 1) * 128],
                                     rhs=h1n[k4][:],
                                     start=(k4 == 0), stop=(k4 == 3))
                h = cst.tile([128, 512], f32r, tag=f"h2{m2}{n}")
                nc.scalar.activation(out=h[:], in_=ph2[:], func=Relu,
                                     bias=cs(f"b2{m2}", f32)[:, :1])
                h2n.append(h)
            po = pml.tile([1, 512], f32, tag="mlp")
            for k2 in range(2):
                nc.tensor.matmul(out=po[:], lhsT=cs(f"w3{k2}")[:, :1],
                                 rhs=h2n[k2][:], start=(k2 == 0), stop=(k2 == 1))
            nc.vector.tensor_scalar_add(out=out_sb[:1, n * 512:(n + 1) * 512],
                                        in0=po[:], scalar1=cs("b3", f32)[:1, :1])
        nc.sync.dma_start(out=out[:], in_=out_sb[:])

    nc.compile()
    return nc


def _pack_consts(Wp, W1, W2, W3, bp, b1, b2, b3, inv, xgt):
    cpk = np.zeros((128, CW), np.float32)
    def put(name, arr):
        o, w = _off[name]
        cpk[:, o:o + w] = arr
    for k in range(2):
        put(f"wp{k}", Wp[k * 128:(k + 1) * 128])
        put(f"w1t{k}", W1[k * 128:(k + 1) * 128])
        put(f"w1b{k}", W1[D + k * 128:D + (k + 1) * 128])
        put(f"w3{k}", W3[k * 128:(k + 1) * 128])
        put(f"bp{k}", bp[k * 128:(k + 1) * 128, None])
        put(f"b2{k}", b2[k * 128:(k + 1) * 128, None])
        put(f"xg{k}", xgt[k * 128:(k + 1) * 128])
    for k in range(4):
        put(f"w2{k}", W2[k * 128:(k + 1) * 128])
        put(f"b1{k}", b1[k * 128:(k + 1) * 128, None])
    cpk[0, _off["b3"][0]] = b3[0]
    put("inv", inv)
    put("ident", np.eye(128, dtype=np.float32))
    return np.ascontiguousarray(cpk)


def kernel(x, edge_attr, batch, target_node_mask, true_nodes_idx,
           Wp, bp, W1, b1, W2, b2, W3, b3,
           num_graphs=G, num_classes=C, **_):
    x = np.ascontiguousarray(np.asarray(x), dtype=np.float32)
    batch = np.asarray(batch).astype(np.int64)
    mask = np.asarray(target_node_mask).astype(bool)
    idx = np.asarray(true_nodes_idx).astype(np.int64)
    Wp = np.asarray(Wp, np.float32)
    W1 = np.asarray(W1, np.float32)
    W2 = np.asarray(W2, np.float32)
    W3 = np.asarray(W3, np.float32)
    bp = np.asarray(bp, np.float32)
    b1 = np.asarray(b1, np.float32)
    b2 = np.asarray(b2, np.float32)
    b3 = np.asarray(b3, np.float32)

    ncount = np.bincount(batch[mask], minlength=G).astype(np.float32)
    with np.errstate(divide="ignore"):
        inv_all = (np.float32(1.0) / ncount).astype(np.float32)

    core = batch // GL
    sel_rows = [np.flatnonzero((core == k) & mask) for k in range(M)]
    NT = max(1, max((len(r) + 127) // 128 for r in sel_rows))

    if NT not in _cache:
        _cache[NT] = _build(NT)
    nc = _cache[NT]

    in_maps = []
    for k in range(M):
        rows = sel_rows[k]
        nk = len(rows)
        xci3 = np.full((NT * 128, FW), -1.0, np.float32)
        xci3[:, :D] = 0.0
        xci3[:nk, :D] = x[rows]
        xci3[:nk, D] = (batch[rows] - k * GL).astype(np.float32)
        xci = np.ascontiguousarray(
            xci3.reshape(NT, 128, FW).transpose(1, 0, 2).reshape(128, NT * FW))
        inv = inv_all[k * GL:(k + 1) * GL].reshape(GL, 1)
        xgt = np.ascontiguousarray(x[idx[k * ROWS:(k + 1) * ROWS]].T)
        cpk = _pack_consts(Wp, W1, W2, W3, bp, b1, b2, b3, inv, xgt)
        iot = np.ascontiguousarray(np.broadcast_to(
            np.arange(GL, dtype=np.float32), (128, GL)))
        in_maps.append(dict(xci=xci, cpk=cpk, iot=iot))

    res = run_bass_kernel_spmd(nc, in_maps, list(range(M)))
    out = np.concatenate([res.results[k]["out"].reshape(ROWS) for k in range(M)])
    return out.reshape(G * C, 1).astype(np.float32)


# revision 10
# speedup vs baseline: 1.1556x; 1.1556x over previous
"""Trainium2 Bass kernel for nn_MultiHeadModel (segment_reduce), 8-core SPMD.

Math (reference):
    xp  = x @ Wp + bp                              # [N, 256]
    class_emb[g] = (sum_{i in g} m_i * xp_i) / n_g # [G, 256]  (segment mean)
    h   = concat(repeat(class_emb, C), xp[idx])    # [G*C, 512]
    out = relu(relu(h@W1+b1)@W2+b2) @ W3 + b3      # [G*C, 1]
(edge_attr projection in the reference is dead code - output never uses it.)

Structure:
  *  sum_g m_i*xp_i = (sum_g m_i*x_i) @ Wp + n_g*bp: the [N,256] projection
     is never materialized. A one-hot indicator matmul does the segment sum
     over raw x rows on the PE; only [128,256] per core gets projected.
  *  batch is sorted -> graphs shard contiguously: core k owns graphs
     [128k,128k+128). Host drops masked-out rows and packs x plus the
     one-hot indicator into one partition-major stream tensor, 4 node
     tiles per DMA.
  *  xp[idx] rows are host-gathered per core (transposed); the device
     projects them. Those matmuls AND the h1 pre-activation matmuls are
     interleaved into the segment-sum stream so the PE never idles (keeps
     the HAM clock un-throttled). Only the class-embedding broadcast-add,
     relu, h2 and the output head run after the stream.
  *  repeat(class_emb, C) @ W1_top == repeat(class_emb @ W1_top, C),
     applied with a step-0 free-dim AP broadcast.
  *  All weights/biases/identity/gather rows arrive in ONE packed const
     DMA ([128, ~7.8k] partition-major, f32 views via bitcast).
  *  Matmuls in float32r (fp32_mode=HIGH single pass, ~2 cyc/row,
     ~1.5e-4 relative error). All activations computed transposed
     (features on partitions) - no transposes except one 128x256 tile.
"""
import numpy as np
from contextlib import ExitStack

import concourse.bacc as bacc
import concourse.mybir as mybir
from concourse.tile import TileContext
from concourse.bass_utils import run_bass_kernel_spmd

M = 8                 # cores
G = 1024              # graphs
C = 16                # classes
GL = G // M           # graphs per core (128)
D = 256
D2 = 512
ROWS = G * C // M     # MLP rows per core (2048)
NCH = ROWS // 512     # 512-wide row chunks (4)
FW = D + 1            # stream row width: 256 x-feats + 1 graph-id (257)
SUP = 4               # node-tiles per stream DMA

f32 = mybir.dt.float32
f32r = mybir.dt.float32r
Relu = mybir.ActivationFunctionType.Relu
Copy = mybir.ActivationFunctionType.Copy

# ---- packed constant layout (columns of a [128, CW] tile) --------------
_off = {}
_c = 0
def _span(name, w):
    global _c
    _off[name] = (_c, w)
    _c += w
for _k in range(2):
    _span(f"wp{_k}", D)
for _k in range(2):
    _span(f"w1t{_k}", D2)
for _k in range(2):
    _span(f"w1b{_k}", D2)
for _k in range(4):
    _span(f"w2{_k}", D)
for _k in range(2):
    _span(f"w3{_k}", 1)
for _k in range(2):
    _span(f"bp{_k}", 1)
for _k in range(4):
    _span(f"b1{_k}", 1)
for _k in range(2):
    _span(f"b2{_k}", 1)
_span("b3", 1)
_span("inv", 1)
_span("ident", 128)
for _k in range(2):
    _span(f"xg{_k}", ROWS)
CW = _c

_cache = {}


def _build(NT):
    NS = (NT + SUP - 1) // SUP
    nc = bacc.Bacc(None, target_bir_lowering=False, debug=False)
    xci = nc.dram_tensor("xci", [128, NT * FW], f32r, kind="ExternalInput")
    iot = nc.dram_tensor("iot", [128, GL], f32r, kind="ExternalInput")
    cpk = nc.dram_tensor("cpk", [128, CW], f32r, kind="ExternalInput")
    out = nc.dram_tensor("out", [1, ROWS], f32, kind="ExternalOutput")

    with TileContext(nc) as tc, ExitStack() as ctx:
        cst = ctx.enter_context(tc.tile_pool(name="cst", bufs=1))
        stream = ctx.enter_context(tc.tile_pool(name="stream", bufs=6))
        pseg = ctx.enter_context(tc.tile_pool(name="pseg", bufs=1, space="PSUM"))
        pmisc = ctx.enter_context(tc.tile_pool(name="pmisc", bufs=2, space="PSUM"))
        pml = ctx.enter_context(tc.tile_pool(name="pml", bufs=4, space="PSUM"))

        # ---- stream head first (segment-sum can start immediately), then
        # the packed const DMA, then the rest of the stream ---------------
        def stream_dma(st):
            t0 = st * SUP
            n_sub = min(SUP, NT - t0)
            stile = stream.tile([128, SUP * FW], f32r, tag="s")
            nc.sync.dma_start(out=stile[:, :n_sub * FW],
                              in_=xci[:, t0 * FW:(t0 + n_sub) * FW])
            return stile, n_sub

        iota_t = cst.tile([128, GL], f32r, tag="iota")
        nc.sync.dma_start(out=iota_t[:], in_=iot[:])

        NSH = min(2, NS)
        head = [stream_dma(st) for st in range(NSH)]

        ctile = cst.tile([128, CW], f32r, tag="cpk")
        nc.sync.dma_start(out=ctile[:], in_=cpk[:])

        def cs(name, dt=f32r):           # const slice
            o, w = _off[name]
            ap = ctile[:, o:o + w]
            return ap.bitcast(dt) if dt is not f32r else ap

        # ---- stream + interleaved MLP-left matmuls ---------------------
        psum_sx = pseg.tile([GL, D], f32)
        xgp = [[None] * NCH for _ in range(2)]
        h1pre = [[None] * NCH for _ in range(4)]

        def xgp_group(mc, n):
            pp = pml.tile([128, 512], f32, tag="mlp")
            for k2 in range(2):
                nc.tensor.matmul(out=pp[:],
                                 lhsT=cs(f"wp{k2}")[:, mc * 128:(mc + 1) * 128],
                                 rhs=cs(f"xg{k2}")[:, n * 512:(n + 1) * 512],
                                 start=(k2 == 0), stop=(k2 == 1))
            t = cst.tile([128, 512], f32r, tag=f"xgp{mc}{n}")
            nc.vector.tensor_scalar_add(out=t[:], in0=pp[:],
                                        scalar1=cs(f"bp{mc}", f32)[:, :1])
            xgp[mc][n] = t

        def h1pre_group(m1, n):
            ph = pml.tile([128, 512], f32, tag="mlp")
            for k2 in range(2):
                nc.tensor.matmul(out=ph[:],
                                 lhsT=cs(f"w1b{k2}")[:, m1 * 128:(m1 + 1) * 128],
                                 rhs=xgp[k2][n][:],
                                 start=(k2 == 0), stop=(k2 == 1))
            t = cst.tile([128, 512], f32, tag=f"h1p{m1}{n}")
            nc.scalar.activation(out=t[:], in_=ph[:], func=Copy)
            h1pre[m1][n] = t

        jobs = []
        for n in range(NCH):
            jobs.append((xgp_group, 0, n))
            jobs.append((xgp_group, 1, n))
            for m1 in range(4):
                jobs.append((h1pre_group, m1, n))
        jobs_iter = iter(jobs)

        for st in range(NS):
            if st < NSH:
                stile, n_sub = head[st]
            else:
                stile, n_sub = stream_dma(st)
            for s in range(n_sub):
                t = st * SUP + s
                # one-hot indicator from the graph-id column (exact 0/1)
                ind_t = stream.tile([128, GL], f32r, tag="ind")
                nc.vector.tensor_tensor(
                    out=ind_t[:],
                    in0=stile[:, s * FW + D:s * FW + D + 1].to_broadcast([128, GL]),
                    in1=iota_t[:],
                    op=mybir.AluOpType.is_equal,
                )
                nc.tensor.matmul(out=psum_sx[:],
                                 lhsT=ind_t[:],
                                 rhs=stile[:, s * FW:s * FW + D],
                                 start=(t == 0), stop=(t == NT - 1))
            if st >= NS - 8:
                for _ in range(3):
                    job = next(jobs_iter, None)
                    if job:
                        job[0](job[1], job[2])
        for job in jobs_iter:
            job[0](job[1], job[2])

        # ---- class-embedding chain (small, latency-bound) --------------
        sxs = cst.tile([GL, D], f32, tag="sxs")
        nc.vector.tensor_scalar_mul(out=sxs[:], in0=psum_sx[:],
                                    scalar1=cs("inv", f32)[:, :1])
        sxT = []
        for c2 in range(2):
            pt = pmisc.tile([128, 128], f32, tag="mm")
            nc.tensor.transpose(out=pt[:], in_=sxs[:, c2 * 128:(c2 + 1) * 128],
                                identity=cs("ident", f32))
            st_ = cst.tile([128, 128], f32r, tag=f"sxT{c2}")
            nc.vector.tensor_copy(out=st_[:], in_=pt[:])
            sxT.append(st_)
        clsembT = []
        for mc in range(2):
            pc = pmisc.tile([128, 128], f32, tag="mm")
            for k2 in range(2):
                nc.tensor.matmul(out=pc[:],
                                 lhsT=cs(f"wp{k2}")[:, mc * 128:(mc + 1) * 128],
                                 rhs=sxT[k2][:], start=(k2 == 0), stop=(k2 == 1))
            ce = cst.tile([128, 128], f32r, tag=f"ce{mc}")
            nc.vector.tensor_scalar_add(out=ce[:], in0=pc[:],
                                        scalar1=cs(f"bp{mc}", f32)[:, :1])
            clsembT.append(ce)
        cls1b = []
        for m1 in range(4):
            p1_ = pmisc.tile([128, 128], f32, tag="mm")
            for k2 in range(2):
                nc.tensor.matmul(out=p1_[:],
                                 lhsT=cs(f"w1t{k2}")[:, m1 * 128:(m1 + 1) * 128],
                                 rhs=clsembT[k2][:], start=(k2 == 0), stop=(k2 == 1))
            cb = cst.tile([128, GL], f32, tag=f"cb{m1}")
            nc.vector.tensor_scalar_add(out=cb[:], in0=p1_[:],
                                        scalar1=cs(f"b1{m1}", f32)[:, :1])
            cls1b.append(cb)

        # ---- tail: h1 = relu(h1pre + cls1b[bcast]), h2, out, pipelined -
        out_sb = cst.tile([1, ROWS], f32, tag="osb")
        for n in range(NCH):
            h1n = []
            for m1 in range(4):
                hp = h1pre[m1][n]
                nc.vector.tensor_tensor(
                    out=hp[:].rearrange("p (g c) -> p g c", c=C),
                    in0=hp[:].rearrange("p (g c) -> p g c", c=C),
                    in1=cls1b[m1][:, n * 32:(n + 1) * 32, None].to_broadcast([128, 32, C]),
                    op=mybir.AluOpType.add,
                )
                h = cst.tile([128, 512], f32r, tag=f"h1{m1}{n}")
                nc.scalar.activation(out=h[:], in_=hp[:], func=Relu)
                h1n.append(h)
            h2n = []
            for m2 in range(2):
                ph2 = pml.tile([128, 512], f32, tag="mlp")
                for k4 in range(4):
                    nc.tensor.matmul(out=ph2[:],
                                     lhsT=cs(f"w2{k4}")[:, m2 * 128:(m2 + 1) * 128],
                                     rhs=h1n[k4][:],
                                     start=(k4 == 0), stop=(k4 == 3))
                h = cst.tile([128, 512], f32r, tag=f"h2{m2}{n}")
                nc.scalar.activation(out=h[:], in_=ph2[:], func=Relu,
                                     bias=cs(f"b2{m2}", f32)[:, :1])
                h2n.append(h)
            po = pml.tile([1, 512], f32, tag="mlp")
            for k2 in range(2):
                nc.tensor.matmul(out=po[:], lhsT=cs(f"w3{k2}")[:, :1],
                                 rhs=h2n[k2][:], start=(k2 == 0), stop=(k2 == 1))
            nc.vector.tensor_scalar_add(out=out_sb[:1, n * 512:(n + 1) * 512],
                                        in0=po[:], scalar1=cs("b3", f32)[:1, :1])
        nc.sync.dma_start(out=out[:], in_=out_sb[:])

    nc.compile()
    return nc


def _pack_consts(Wp, W1, W2, W3, bp, b1, b2, b3, inv, xgt):
    cpk = np.zeros((128, CW), np.float32)
    def put(name, arr):
        o, w = _off[name]
        cpk[:, o:o + w] = arr
    for k in range(2):
        put(f"wp{k}", Wp[k * 128:(k + 1) * 128])
        put(f"w1t{k}", W1[k * 128:(k + 1) * 128])
        put(f"w1b{k}", W1[D + k * 128:D + (k + 1) * 128])
        put(f"w3{k}", W3[k * 128:(k + 1) * 128])
        put(f"bp{k}", bp[k * 128:(k + 1) * 128, None])
        put(f"b2{k}", b2[k * 128:(k + 1) * 128, None])
        put(f"xg{k}", xgt[k * 128:(k + 1) * 128])
    for k in range(4):
        put(f"w2{k}", W2[k * 128:(k + 1) * 128])
        put(f"b1{k}", b1[k * 128:(k + 1) * 128, None])
    cpk[0, _off["b3"][0]] = b3[0]
    put("inv", inv)
    put("ident", np.eye(128, dtype=np.float32))
    return np.ascontiguousarray(cpk)


def kernel(x, edge_attr, batch, target_node_mask, true_nodes_idx,
           Wp, bp, W1, b1, W2, b2, W3, b3,
           num_graphs=G, num_classes=C, **_):
    x = np.ascontiguousarray(np.asarray(x), dtype=np.float32)
    batch = np.asarray(batch).astype(np.int64)
    mask = np.asarray(target_node_mask).astype(bool)
    idx = np.asarray(true_nodes_idx).astype(np.int64)
    Wp = np.asarray(Wp, np.float32)
    W1 = np.asarray(W1, np.float32)
    W2 = np.asarray(W2, np.float32)
    W3 = np.asarray(W3, np.float32)
    bp = np.asarray(bp, np.float32)
    b1 = np.asarray(b1, np.float32)
    b2 = np.asarray(b2, np.float32)
    b3 = np.asarray(b3, np.float32)

    ncount = np.bincount(batch[mask], minlength=G).astype(np.float32)
    with np.errstate(divide="ignore"):
        inv_all = (np.float32(1.0) / ncount).astype(np.float32)

    core = batch // GL
    sel_rows = [np.flatnonzero((core == k) & mask) for k in range(M)]
    NT = max(1, max((len(r) + 127) // 128 for r in sel_rows))

    if NT not in _cache:
        _cache[NT] = _build(NT)
    nc = _cache[NT]

    in_maps = []
    for k in range(M):
        rows = sel_rows[k]
        nk = len(rows)
        xci3 = np.full((NT * 128, FW), -1.0, np.float32)
        xci3[:, :D] = 0.0
        xci3[:nk, :D] = x[rows]
        xci3[:nk, D] = (batch[rows] - k * GL).astype(np.float32)
        xci = np.ascontiguousarray(
            xci3.reshape(NT, 128, FW).transpose(1, 0, 2).reshape(128, NT * FW))
        inv = inv_all[k * GL:(k + 1) * GL].reshape(GL, 1)
        xgt = np.ascontiguousarray(x[idx[k * ROWS:(k + 1) * ROWS]].T)
        cpk = _pack_consts(Wp, W1, W2, W3, bp, b1, b2, b3, inv, xgt)
        iot = np.ascontiguousarray(np.broadcast_to(
            np.arange(GL, dtype=np.float32), (128, GL)))
        in_maps.append(dict(xci=xci, cpk=cpk, iot=iot))

    res = run_bass_kernel_spmd(nc, in_maps, list(range(M)))
    out = np.concatenate([res.results[k]["out"].reshape(ROWS) for k in range(M)])
    return out.reshape(G * C, 1).astype(np.float32)
